# revision 27
# baseline (speedup 1.0000x reference)
"""fp8 DoubleRow matmul; drain rebalanced to fit Act+DVE capacity.

Each core computes its A-row slab sim [1536, 12288] in fp8e4 DoubleRow mode
(256-deep contraction per instruction). Inputs are scaled by 32 so fp8e4
covers the descriptor range; the scale cancels in ranking.

Schedule (175.2us -> ~150us):
- PSUM tiles are single-m [128, 1024] (2 banks) with bufs=4 instead of
  [128, 2, 1024] pairs with bufs=2: the psum-reuse window grows from one
  fill period (1720ns) to three (2580ns), absorbing drain jitter.
- The device only reduces each 1024-col chunk slab to 512 HALF window
  maxima (col w pairs with w+512); the host folds the remaining pair.
  This halves per-tile DVE work so Act (8 copy-drains/chunk, ~86%) and
  DVE (4 psum-direct reduces + 8 sbuf maxes, ~81%) both fit under the PE
  fill rate — a full on-device reduction sits at ~100% of combined
  Act+DVE capacity and stalls the PE.
- wm writes go through the otherwise-idle SP hardware DGE, batched per
  4 m-tiles, keeping Act/DVE free of DMA trigger overhead.
- Startup streams on parallel queues (SP: both lhsT k-planes, Act: rhs
  chunk-0 half 0, Pool swdge: half 1); rhs is loaded in 512-col halves
  and chunk 0 runs h-major (all four tiles' h0 half-groups first), so
  matmuls start as soon as ~790KB has landed and rhs half 1's flight
  time is fully hidden behind h0 work.
- The last chunk ships its final tile group as two half writes on two
  queues (SP + Act) to shorten the post-matmul tail.

The host picks the top-8 windows per (row, chunk), exactly rescores all
candidate columns in fp32, and reconstructs both match directions (per-row
top-2 directly; per-column top-2 by scattering the same candidates, which
provably contain every column's top-2). Final top-2/ratio/mutual-check math
is fp32, matching the reference.
"""
import sys

sys.path.insert(0, '/opt/trn_rl_repo')

import numpy as np
import ml_dtypes

CH = 512
N1 = 96 * 128
N2 = 96 * 128
N_CORES = 8
SLAB = N1 // N_CORES          # 1536
M_TILES = SLAB // 128         # 12
KT2 = CH // 256               # 2 DoubleRow k-tiles
CB = 1024                     # chunk width
NCB = N2 // CB                # 12
NW = 256                      # host-side windows per chunk
W = CB // NW                  # 4 cols per window
NWH = 512                     # device ships half-reduced maxima
FP8_SCALE = 32.0
RATIO = 0.95
EPS = 1e-8

_compiled = None
LAST_EXEC_NS = None
LAST_RESULTS = None


def _build():
    import concourse.bacc as bacc
    import concourse.tile as tile
    from concourse import mybir

    nc = bacc.Bacc("TRN2", target_bir_lowering=False, debug=False,
                   num_devices=N_CORES)

    lhsT_d = nc.dram_tensor("lhsT", [CH, SLAB], mybir.dt.float8e4,
                            kind="ExternalInput")
    rhs_d = nc.dram_tensor("rhs", [CH, N2], mybir.dt.float8e4,
                           kind="ExternalInput")
    wm_d = nc.dram_tensor("wm", [NCB, M_TILES, 128, NWH],
                          mybir.dt.bfloat16, kind="ExternalOutput")

    with tile.TileContext(nc) as tc:
        with tc.tile_pool(name="lhs", bufs=1) as lhs_pool, \
             tc.tile_pool(name="rhs", bufs=3) as rhs_pool, \
             tc.tile_pool(name="scopy", bufs=4) as s_pool, \
             tc.tile_pool(name="wout", bufs=3) as w_pool, \
             tc.tile_pool(name="ps", bufs=4, space="PSUM") as ps_pool:
            # lhs split per k-tile, both on the SP hw queue (k1 lands right
            # behind k0, before the first k1 matmul needs it)
            lhk = []
            for k in range(KT2):
                t = lhs_pool.tile([128, 2, SLAB], mybir.dt.float8e4,
                                  tag=f"lh{k}")
                nc.sync.dma_start(
                    out=t[:],
                    in_=lhsT_d.ap()[k * 256:(k + 1) * 256]
                    .rearrange("(two p) m -> p two m", p=128, two=2))
                lhk.append(t)

            def load_rh(cb, engs):
                # two independent half tiles so matmuls on half 0 don't
                # wait for half 1's DMA; at startup the halves go to two
                # different queues so they stream in parallel
                halves = []
                for h in range(2):
                    rh = rhs_pool.tile([128, KT2, 2, 512],
                                       mybir.dt.float8e4, tag=f"rh{h}")
                    c0 = cb * CB + h * 512
                    engs[h].dma_start(
                        out=rh[:],
                        in_=rhs_d.ap()[:, c0:c0 + 512]
                        .rearrange("(kt two p) n -> p kt two n",
                                   p=128, two=2))
                    halves.append(rh)
                return halves

            # Light PE warmup on memset scratch: PE activity from ~8.6us
            # burns the power-ramp window while input DMAs are in flight.
            # Four ps-pool allocations keep the slot rotation aligned.
            wl = lhs_pool.tile([128, 2, 128], mybir.dt.float8e4,
                               tag="warm_l")
            wr = lhs_pool.tile([128, 2, 512], mybir.dt.float8e4,
                               tag="warm_r")
            nc.vector.memset(wl[:], 0)
            nc.vector.memset(wr[:], 0)
            for _ in range(4):
                wp = ps_pool.tile([128, CB], mybir.dt.float32, tag="ps")
                for i in range(2):
                    nc.tensor.matmul(
                        out=wp[:, i * 512:(i + 1) * 512],
                        lhsT=wl[:], rhs=wr[:], start=True, stop=True,
                        perf_mode=mybir.MatmulPerfMode.DoubleRow)

            rh = load_rh(0, (nc.scalar, nc.gpsimd))
            w4 = None

            def drain(cb, m, ps):
                # 7 Act copies + 5 DVE reduces per chunk: evenly
                # interleaved so the Act queue never backs up two-deep
                # (an adjacent A-pair leaves the second copy's psum slot
                # release ~2.2us after tile end, against a 2.58us window)
                is_v = m in (2, 4, 6, 8, 10)
                if is_v:
                    nc.vector.tensor_reduce(
                        out=w4[:, m % 4],
                        in_=ps.rearrange("p (j w) -> p w j", j=2, w=512),
                        axis=mybir.AxisListType.X, op=mybir.AluOpType.max)
                else:
                    s = s_pool.tile([128, CB], mybir.dt.bfloat16, tag="s")
                    nc.scalar.copy(s[:], ps[:])
                    nc.vector.tensor_max(w4[:, m % 4], s[:, :512],
                                         s[:, 512:])

            for cb in range(NCB):
                rh_next = (load_rh(cb + 1, (nc.sync, nc.sync))
                           if cb + 1 < NCB else None)

                m0 = 0
                if cb == 0:
                    # h-major start: the h0 half-groups of four tiles only
                    # need rh half 0, hiding rh half 1's in-flight time
                    m0 = 4
                    ps_g = []
                    for mg in range(4):
                        ps = ps_pool.tile([128, CB], mybir.dt.float32,
                                          tag="ps")
                        ps_g.append(ps)
                    for mg in range(4):
                        for k in range(KT2):
                            nc.tensor.matmul(
                                out=ps_g[mg][:, 0:512],
                                lhsT=lhk[k][:, :, mg * 128:(mg + 1) * 128],
                                rhs=rh[0][:, k],
                                start=(k == 0), stop=(k == KT2 - 1),
                                perf_mode=mybir.MatmulPerfMode.DoubleRow)
                    w4 = w_pool.tile([128, 4, NWH], mybir.dt.bfloat16,
                                     tag="w4")
                    for mg in range(4):
                        for k in range(KT2):
                            nc.tensor.matmul(
                                out=ps_g[mg][:, 512:1024],
                                lhsT=lhk[k][:, :, mg * 128:(mg + 1) * 128],
                                rhs=rh[1][:, k],
                                start=(k == 0), stop=(k == KT2 - 1),
                                perf_mode=mybir.MatmulPerfMode.DoubleRow)
                        drain(cb, mg, ps_g[mg])
                    nc.sync.dma_start(
                        out=wm_d.ap()[cb, 0:4]
                        .rearrange("m p w -> p m w"),
                        in_=w4[:])

                for m in range(m0, M_TILES):
                    msl = slice(m * 128, (m + 1) * 128)
                    ps = ps_pool.tile([128, CB], mybir.dt.float32, tag="ps")
                    for h in range(2):
                        for k in range(KT2):
                            nc.tensor.matmul(
                                out=ps[:, h * 512:(h + 1) * 512],
                                lhsT=lhk[k][:, :, msl],
                                rhs=rh[h][:, k],
                                start=(k == 0),
                                stop=(k == KT2 - 1),
                                perf_mode=mybir.MatmulPerfMode.DoubleRow)
                    if m % 4 == 0:
                        w4 = w_pool.tile([128, 4, NWH], mybir.dt.bfloat16,
                                         tag="w4")
                    drain(cb, m, ps)
                    if cb == NCB - 1 and m == 9:
                        # tail: ship the last group in two halves on two
                        # queues so the final transfer is half-sized
                        nc.sync.dma_start(
                            out=wm_d.ap()[cb, 8:10]
                            .rearrange("m p w -> p m w"),
                            in_=w4[:, :2])
                    elif cb == NCB - 1 and m == 11:
                        nc.scalar.dma_start(
                            out=wm_d.ap()[cb, 10:12]
                            .rearrange("m p w -> p m w"),
                            in_=w4[:, 2:])
                    elif m % 4 == 3:
                        nc.sync.dma_start(
                            out=wm_d.ap()[cb, m - 3:m + 1]
                            .rearrange("m p w -> p m w"),
                            in_=w4[:])
                rh = rh_next

    nc.compile()
    return nc


def _get_compiled():
    global _compiled
    if _compiled is None:
        _compiled = _build()
    return _compiled


def _normalize(fmap):
    d = np.asarray(fmap).reshape(CH, -1).astype(np.float32)
    nrm = np.sqrt(np.sum(np.square(d), axis=0, keepdims=True,
                         dtype=np.float32))
    return (d / nrm).astype(np.float32)


def _install_trace_shim():
    import types

    try:
        import antenv.axon_hooks  # noqa: F401
    except ImportError:
        from trn_agent_boot.trn_boot import _ntff_profile_via_ctypes
        hook = _ntff_profile_via_ctypes('/opt/axon/libaxon_pjrt.so')
        mod = types.ModuleType('antenv.axon_hooks')
        mod.get_axon_ntff_profile_hook = lambda: hook
        mod.set_axon_ntff_profile_hook = lambda h: None
        sys.modules['antenv.axon_hooks'] = mod
    import concourse.bass_utils as bu
    bu.upload_artifacts = lambda tmpdir: tmpdir


def kernel(map_A, map_B):
    import os

    from concourse.bass_utils import run_bass_kernel_spmd

    global LAST_EXEC_NS, LAST_RESULTS
    trace = bool(int(os.environ.get("KERNEL_TRACE", "0")))
    if trace:
        _install_trace_shim()
    nc = _get_compiled()

    nA = _normalize(map_A)            # [CH, N1] unit cols
    nB = _normalize(map_B)            # [CH, N2]
    f8 = ml_dtypes.float8_e4m3
    nAf = (nA * np.float32(FP8_SCALE)).astype(f8)
    nBf = np.ascontiguousarray((nB * np.float32(FP8_SCALE)).astype(f8))

    in_maps = []
    for c in range(N_CORES):
        sl = slice(c * SLAB, (c + 1) * SLAB)
        in_maps.append({
            "lhsT": np.ascontiguousarray(nAf[:, sl]),
            "rhs": nBf,
        })

    res = run_bass_kernel_spmd(nc, in_maps, core_ids=list(range(N_CORES)),
                               trace=trace)
    LAST_EXEC_NS = res.exec_time_ns
    LAST_RESULTS = res

    # Half-reduced window maxima [N1, NCB, 512]: fold the pair on host,
    # then pick top-8 windows per (row, chunk).
    wmh = np.concatenate(
        [res.results[c]["wm"].transpose(1, 2, 0, 3).reshape(SLAB, NCB, NWH)
         for c in range(N_CORES)]).astype(np.float32)
    wmr = np.maximum(wmh[:, :, :NW], wmh[:, :, NW:])    # [N1, NCB, NW]
    widx = np.argpartition(-wmr, 8, axis=2)[:, :, :8].astype(np.int64)
    choff = (np.arange(NCB, dtype=np.int64) * CB)[None, :, None]
    wcol = widx + choff                                 # window base col
    cols = (wcol[..., None] + (np.arange(W, dtype=np.int64) * NW)
            [None, None, None, :]).reshape(N1, NCB * 8 * W)   # [N1, K]
    K = cols.shape[1]

    # Exact rescoring of every candidate pair in fp32.
    d1 = nA.T                                           # [N1, CH]
    d2 = nB.T                                           # [N2, CH]
    E = np.empty((N1, K), np.float32)
    BS = 512
    for s in range(0, N1, BS):
        g = d2[cols[s:s + BS]]                          # [bs, K, CH]
        E[s:s + BS] = np.matmul(
            g, d1[s:s + BS, :, None], dtype=np.float32)[..., 0]

    # Direction 1: exact top-2 per row.
    p3 = np.argpartition(-E, 2, axis=1)[:, :3]
    v3 = np.take_along_axis(E, p3, 1)
    c3 = np.take_along_axis(cols, p3, 1)
    o3 = np.lexsort((c3, -v3), axis=1)
    v3 = np.take_along_axis(v3, o3, 1)
    c3 = np.take_along_axis(c3, o3, 1)
    m1_12 = v3[:, 0]
    m2_12 = v3[:, 1]
    nn12 = c3[:, 0]

    # Direction 2: per-column top-2 from the scattered candidates.
    r_flat = np.repeat(np.arange(N1, dtype=np.int64), K)
    c_flat = cols.ravel()
    v_flat = E.ravel()
    order = np.lexsort((r_flat, -v_flat, c_flat))
    cs = c_flat[order]
    vs = v_flat[order]
    rs = r_flat[order]
    starts = np.searchsorted(cs, np.arange(N2, dtype=np.int64), 'left')
    ends = np.searchsorted(cs, np.arange(N2, dtype=np.int64), 'right')
    cnt = ends - starts
    m1_21 = np.full(N2, -1.0, np.float32)
    m2_21 = np.full(N2, -1.0, np.float32)
    nn21 = np.zeros(N2, np.int64)
    has1 = cnt >= 1
    m1_21[has1] = vs[starts[has1]]
    nn21[has1] = rs[starts[has1]]
    has2 = cnt >= 2
    m2_21[has2] = vs[starts[has2] + 1]

    two = np.float32(2.0)
    ratios12 = (two - two * m1_12) / ((two - two * m2_12) + np.float32(EPS))
    ratios21 = (two - two * m1_21) / ((two - two * m2_21) + np.float32(EPS))

    ids1 = np.arange(N1)
    mask = ((ids1 == nn21[nn12]) & (ratios12 <= np.float32(RATIO))
            & (ratios21[nn12] <= np.float32(RATIO)))
    masked_sim = np.where(mask, m1_12, 0.0).astype(np.float32)
    return masked_sim, nn12.astype(np.int32), mask


# revision 28
# speedup vs baseline: 1.0181x; 1.0181x over previous
"""fp8 DoubleRow matmul; drain rebalanced to fit Act+DVE capacity.

Each core computes its A-row slab sim [1536, 12288] in fp8e4 DoubleRow mode
(256-deep contraction per instruction). Inputs are scaled by 32 so fp8e4
covers the descriptor range; the scale cancels in ranking.

Schedule (175.2us -> ~150us):
- PSUM tiles are single-m [128, 1024] (2 banks) with bufs=4 instead of
  [128, 2, 1024] pairs with bufs=2: the psum-reuse window grows from one
  fill period (1720ns) to three (2580ns), absorbing drain jitter.
- The device only reduces each 1024-col chunk slab to 512 HALF window
  maxima (col w pairs with w+512); the host folds the remaining pair.
  This halves per-tile DVE work so Act (8 copy-drains/chunk, ~86%) and
  DVE (4 psum-direct reduces + 8 sbuf maxes, ~81%) both fit under the PE
  fill rate — a full on-device reduction sits at ~100% of combined
  Act+DVE capacity and stalls the PE.
- wm writes go through the otherwise-idle SP hardware DGE, batched per
  4 m-tiles, keeping Act/DVE free of DMA trigger overhead.
- Startup streams on parallel queues (SP: both lhsT k-planes, Act: rhs
  chunk-0 half 0, Pool swdge: half 1); rhs is loaded in 512-col halves
  and chunk 0 runs h-major (all four tiles' h0 half-groups first), so
  matmuls start as soon as ~790KB has landed and rhs half 1's flight
  time is fully hidden behind h0 work.
- The last chunk ships its final tile group as two half writes on two
  queues (SP + Act) to shorten the post-matmul tail.

The host picks the top-8 windows per (row, chunk), exactly rescores all
candidate columns in fp32, and reconstructs both match directions (per-row
top-2 directly; per-column top-2 by scattering the same candidates, which
provably contain every column's top-2). Final top-2/ratio/mutual-check math
is fp32, matching the reference.
"""
import sys

sys.path.insert(0, '/opt/trn_rl_repo')

import numpy as np
import ml_dtypes

CH = 512
N1 = 96 * 128
N2 = 96 * 128
N_CORES = 8
SLAB = N1 // N_CORES          # 1536
M_TILES = SLAB // 128         # 12
KT2 = CH // 256               # 2 DoubleRow k-tiles
CB = 1024                     # chunk width
NCB = N2 // CB                # 12
NW = 256                      # host-side windows per chunk
W = CB // NW                  # 4 cols per window
NWH = 512                     # device ships half-reduced maxima
FP8_SCALE = 32.0
RATIO = 0.95
EPS = 1e-8

_compiled = None
LAST_EXEC_NS = None
LAST_RESULTS = None


def _build():
    import concourse.bacc as bacc
    import concourse.tile as tile
    from concourse import mybir

    nc = bacc.Bacc("TRN2", target_bir_lowering=False, debug=False,
                   num_devices=N_CORES)

    lhsT_d = nc.dram_tensor("lhsT", [CH, SLAB], mybir.dt.float8e4,
                            kind="ExternalInput")
    rhs_d = nc.dram_tensor("rhs", [CH, N2], mybir.dt.float8e4,
                           kind="ExternalInput")
    wm_d = nc.dram_tensor("wm", [NCB, M_TILES, 128, NWH],
                          mybir.dt.bfloat16, kind="ExternalOutput")

    with tile.TileContext(nc) as tc:
        with tc.tile_pool(name="lhs", bufs=1) as lhs_pool, \
             tc.tile_pool(name="rhs", bufs=3) as rhs_pool, \
             tc.tile_pool(name="scopy", bufs=4) as s_pool, \
             tc.tile_pool(name="wout", bufs=3) as w_pool, \
             tc.tile_pool(name="ps", bufs=4, space="PSUM") as ps_pool:
            # lhs split per k-tile, both on the SP hw queue (k1 lands right
            # behind k0, before the first k1 matmul needs it)
            lhk = []
            for k in range(KT2):
                t = lhs_pool.tile([128, 2, SLAB], mybir.dt.float8e4,
                                  tag=f"lh{k}")
                nc.sync.dma_start(
                    out=t[:],
                    in_=lhsT_d.ap()[k * 256:(k + 1) * 256]
                    .rearrange("(two p) m -> p two m", p=128, two=2))
                lhk.append(t)

            def load_rh(cb, engs):
                # two independent half tiles so matmuls on half 0 don't
                # wait for half 1's DMA; at startup the halves go to two
                # different queues so they stream in parallel
                halves = []
                for h in range(2):
                    rh = rhs_pool.tile([128, KT2, 2, 512],
                                       mybir.dt.float8e4, tag=f"rh{h}")
                    c0 = cb * CB + h * 512
                    engs[h].dma_start(
                        out=rh[:],
                        in_=rhs_d.ap()[:, c0:c0 + 512]
                        .rearrange("(kt two p) n -> p kt two n",
                                   p=128, two=2))
                    halves.append(rh)
                return halves

            rh = load_rh(0, (nc.scalar, nc.gpsimd))
            w4 = None

            def drain(cb, m, ps):
                # 7 Act copies + 5 DVE reduces per chunk: evenly
                # interleaved so the Act queue never backs up two-deep
                # (an adjacent A-pair leaves the second copy's psum slot
                # release ~2.2us after tile end, against a 2.58us window)
                is_v = m in (2, 4, 6, 8, 10)
                if is_v:
                    nc.vector.tensor_reduce(
                        out=w4[:, m % 4],
                        in_=ps.rearrange("p (j w) -> p w j", j=2, w=512),
                        axis=mybir.AxisListType.X, op=mybir.AluOpType.max)
                else:
                    s = s_pool.tile([128, CB], mybir.dt.bfloat16, tag="s")
                    nc.scalar.copy(s[:], ps[:])
                    nc.vector.tensor_max(w4[:, m % 4], s[:, :512],
                                         s[:, 512:])

            for cb in range(NCB):
                rh_next = (load_rh(cb + 1, (nc.sync, nc.sync))
                           if cb + 1 < NCB else None)

                m0 = 0
                if cb == 0:
                    # h-major start: the h0 half-groups of four tiles only
                    # need rh half 0, hiding rh half 1's in-flight time
                    m0 = 4
                    ps_g = []
                    for mg in range(4):
                        ps = ps_pool.tile([128, CB], mybir.dt.float32,
                                          tag="ps")
                        ps_g.append(ps)
                    for mg in range(4):
                        for k in range(KT2):
                            nc.tensor.matmul(
                                out=ps_g[mg][:, 0:512],
                                lhsT=lhk[k][:, :, mg * 128:(mg + 1) * 128],
                                rhs=rh[0][:, k],
                                start=(k == 0), stop=(k == KT2 - 1),
                                perf_mode=mybir.MatmulPerfMode.DoubleRow)
                    w4 = w_pool.tile([128, 4, NWH], mybir.dt.bfloat16,
                                     tag="w4")
                    for mg in range(4):
                        for k in range(KT2):
                            nc.tensor.matmul(
                                out=ps_g[mg][:, 512:1024],
                                lhsT=lhk[k][:, :, mg * 128:(mg + 1) * 128],
                                rhs=rh[1][:, k],
                                start=(k == 0), stop=(k == KT2 - 1),
                                perf_mode=mybir.MatmulPerfMode.DoubleRow)
                        drain(cb, mg, ps_g[mg])
                    nc.sync.dma_start(
                        out=wm_d.ap()[cb, 0:4]
                        .rearrange("m p w -> p m w"),
                        in_=w4[:])

                for m in range(m0, M_TILES):
                    msl = slice(m * 128, (m + 1) * 128)
                    ps = ps_pool.tile([128, CB], mybir.dt.float32, tag="ps")
                    for h in range(2):
                        for k in range(KT2):
                            nc.tensor.matmul(
                                out=ps[:, h * 512:(h + 1) * 512],
                                lhsT=lhk[k][:, :, msl],
                                rhs=rh[h][:, k],
                                start=(k == 0),
                                stop=(k == KT2 - 1),
                                perf_mode=mybir.MatmulPerfMode.DoubleRow)
                    if m % 4 == 0:
                        w4 = w_pool.tile([128, 4, NWH], mybir.dt.bfloat16,
                                         tag="w4")
                    drain(cb, m, ps)
                    if cb == NCB - 1 and m == 9:
                        # tail: ship the last group in two halves on two
                        # queues so the final transfer is half-sized
                        nc.sync.dma_start(
                            out=wm_d.ap()[cb, 8:10]
                            .rearrange("m p w -> p m w"),
                            in_=w4[:, :2])
                    elif cb == NCB - 1 and m == 11:
                        nc.scalar.dma_start(
                            out=wm_d.ap()[cb, 10:12]
                            .rearrange("m p w -> p m w"),
                            in_=w4[:, 2:])
                    elif m % 4 == 3:
                        nc.sync.dma_start(
                            out=wm_d.ap()[cb, m - 3:m + 1]
                            .rearrange("m p w -> p m w"),
                            in_=w4[:])
                rh = rh_next

    nc.compile()
    return nc


def _get_compiled():
    global _compiled
    if _compiled is None:
        _compiled = _build()
    return _compiled


def _normalize(fmap):
    d = np.asarray(fmap).reshape(CH, -1).astype(np.float32)
    nrm = np.sqrt(np.sum(np.square(d), axis=0, keepdims=True,
                         dtype=np.float32))
    return (d / nrm).astype(np.float32)


def _install_trace_shim():
    import types

    try:
        import antenv.axon_hooks  # noqa: F401
    except ImportError:
        from trn_agent_boot.trn_boot import _ntff_profile_via_ctypes
        hook = _ntff_profile_via_ctypes('/opt/axon/libaxon_pjrt.so')
        mod = types.ModuleType('antenv.axon_hooks')
        mod.get_axon_ntff_profile_hook = lambda: hook
        mod.set_axon_ntff_profile_hook = lambda h: None
        sys.modules['antenv.axon_hooks'] = mod
    import concourse.bass_utils as bu
    bu.upload_artifacts = lambda tmpdir: tmpdir


def kernel(map_A, map_B):
    import os

    from concourse.bass_utils import run_bass_kernel_spmd

    global LAST_EXEC_NS, LAST_RESULTS
    trace = bool(int(os.environ.get("KERNEL_TRACE", "0")))
    if trace:
        _install_trace_shim()
    nc = _get_compiled()

    nA = _normalize(map_A)            # [CH, N1] unit cols
    nB = _normalize(map_B)            # [CH, N2]
    f8 = ml_dtypes.float8_e4m3
    nAf = (nA * np.float32(FP8_SCALE)).astype(f8)
    nBf = np.ascontiguousarray((nB * np.float32(FP8_SCALE)).astype(f8))

    in_maps = []
    for c in range(N_CORES):
        sl = slice(c * SLAB, (c + 1) * SLAB)
        in_maps.append({
            "lhsT": np.ascontiguousarray(nAf[:, sl]),
            "rhs": nBf,
        })

    res = run_bass_kernel_spmd(nc, in_maps, core_ids=list(range(N_CORES)),
                               trace=trace)
    LAST_EXEC_NS = res.exec_time_ns
    LAST_RESULTS = res

    # Half-reduced window maxima [N1, NCB, 512]: fold the pair on host,
    # then pick top-8 windows per (row, chunk).
    wmh = np.concatenate(
        [res.results[c]["wm"].transpose(1, 2, 0, 3).reshape(SLAB, NCB, NWH)
         for c in range(N_CORES)]).astype(np.float32)
    wmr = np.maximum(wmh[:, :, :NW], wmh[:, :, NW:])    # [N1, NCB, NW]
    widx = np.argpartition(-wmr, 8, axis=2)[:, :, :8].astype(np.int64)
    choff = (np.arange(NCB, dtype=np.int64) * CB)[None, :, None]
    wcol = widx + choff                                 # window base col
    cols = (wcol[..., None] + (np.arange(W, dtype=np.int64) * NW)
            [None, None, None, :]).reshape(N1, NCB * 8 * W)   # [N1, K]
    K = cols.shape[1]

    # Exact rescoring of every candidate pair in fp32.
    d1 = nA.T                                           # [N1, CH]
    d2 = nB.T                                           # [N2, CH]
    E = np.empty((N1, K), np.float32)
    BS = 512
    for s in range(0, N1, BS):
        g = d2[cols[s:s + BS]]                          # [bs, K, CH]
        E[s:s + BS] = np.matmul(
            g, d1[s:s + BS, :, None], dtype=np.float32)[..., 0]

    # Direction 1: exact top-2 per row.
    p3 = np.argpartition(-E, 2, axis=1)[:, :3]
    v3 = np.take_along_axis(E, p3, 1)
    c3 = np.take_along_axis(cols, p3, 1)
    o3 = np.lexsort((c3, -v3), axis=1)
    v3 = np.take_along_axis(v3, o3, 1)
    c3 = np.take_along_axis(c3, o3, 1)
    m1_12 = v3[:, 0]
    m2_12 = v3[:, 1]
    nn12 = c3[:, 0]

    # Direction 2: per-column top-2 from the scattered candidates.
    r_flat = np.repeat(np.arange(N1, dtype=np.int64), K)
    c_flat = cols.ravel()
    v_flat = E.ravel()
    order = np.lexsort((r_flat, -v_flat, c_flat))
    cs = c_flat[order]
    vs = v_flat[order]
    rs = r_flat[order]
    starts = np.searchsorted(cs, np.arange(N2, dtype=np.int64), 'left')
    ends = np.searchsorted(cs, np.arange(N2, dtype=np.int64), 'right')
    cnt = ends - starts
    m1_21 = np.full(N2, -1.0, np.float32)
    m2_21 = np.full(N2, -1.0, np.float32)
    nn21 = np.zeros(N2, np.int64)
    has1 = cnt >= 1
    m1_21[has1] = vs[starts[has1]]
    nn21[has1] = rs[starts[has1]]
    has2 = cnt >= 2
    m2_21[has2] = vs[starts[has2] + 1]

    two = np.float32(2.0)
    ratios12 = (two - two * m1_12) / ((two - two * m2_12) + np.float32(EPS))
    ratios21 = (two - two * m1_21) / ((two - two * m2_21) + np.float32(EPS))

    ids1 = np.arange(N1)
    mask = ((ids1 == nn21[nn12]) & (ratios12 <= np.float32(RATIO))
            & (ratios21[nn12] <= np.float32(RATIO)))
    masked_sim = np.where(mask, m1_12, 0.0).astype(np.float32)
    return masked_sim, nn12.astype(np.int32), mask
